# revision 1
# baseline (speedup 1.0000x reference)
"""Distributed causal attention head for TRN2 (8 NeuronCores).

Problem: B=4, S=4096, D=1024, H=64 fp32.
  q,k,v = x @ W{q,k,v}; scores = q k^T / sqrt(H); causal softmax; out = P v.

Sharding (fully SPMD-uniform, one NEFF for all 8 cores):
  - 4 batches x 2 cores per batch (pair replica groups [[0,1],[2,3],[4,5],[6,7]]).
  - Within a pair, the KEY dimension is split by interleaved 128-row chunks:
    core g owns global key chunks {2i+g}. Each core computes K^T/V (and its
    share of Q^T) from only its own 2048 input rows.
  - Q^T is pair-AllGathered (every core needs all 4096 query columns).
  - Both cores process ALL 8 query blocks of 512; for q-block t each core has
    exactly 2(t+1) local causal key chunks -> identical instruction sequence
    on every core. Causal masking inside the two diagonal chunks uses
    per-core 0/1 mask inputs (multiplicative, after exp).
  - Per-core partial (numerator | denominator) = [4096, 65] fp32 is merged
    with a pair ReduceScatter(add); core g keeps rows [2048g, 2048(g+1)).
    Host divides num/den and re-assembles.

Compute layout notes:
  - All matmuls contract on the partition dim. Scores are computed
    transposed: S_T[k, q] = matmul(lhsT=K^T[:, kchunk], rhs=Q^T[:, qblock]).
  - exp runs on the scalar engine straight out of PSUM (scale=1/8 fused).
  - V is augmented with a ones column, so the AV matmul also produces the
    softmax denominator for free (row 64 of the [65, 512] accumulator).
  - bf16 operands everywhere on the PE (f32 PSUM accumulation); input is
    cast to bf16 on the host, and x^T is loaded via DMA transpose.
"""

import sys

sys.path.insert(0, "/opt/trn_rl_repo")

import numpy as np
import ml_dtypes

B, S, D, H = 4, 4096, 1024, 64
RPC = S // 2            # rows (keys/queries) owned per core
QB = 512                # query block width
NQB = S // QB           # 8 query blocks
NKC = RPC // 128        # 16 local key chunks
NSG = RPC // QB         # 4 own-row groups for projection
BF16 = ml_dtypes.bfloat16
PAIRS = [[0, 1], [2, 3], [4, 5], [6, 7]]

_CACHE = {}


def _build():
    import concourse.bass as bass
    import concourse.mybir as mybir
    from concourse import bacc, tile
    from concourse.masks import make_identity
    from concourse.bass import ts

    f32 = mybir.dt.float32
    bf16 = mybir.dt.bfloat16
    Alu = mybir.AluOpType
    Act = mybir.ActivationFunctionType

    nc = bacc.Bacc(None, target_bir_lowering=False)

    x_ext = nc.declare_dram_parameter("x", [RPC, D], bf16, isOutput=False)
    wqk_ext = nc.declare_dram_parameter("wqk", [D, 128], bf16, isOutput=False)
    wv_ext = nc.declare_dram_parameter("wv", [D, H], bf16, isOutput=False)
    mask_ext = nc.declare_dram_parameter("mask", [2, 128, QB], bf16, isOutput=False)
    out_ext = nc.declare_dram_parameter("out", [RPC, H + 1], f32, isOutput=True)

    with tile.TileContext(nc) as tc:
        with (
            tc.tile_pool(name="persist", bufs=1) as persist,
            tc.tile_pool(name="dram", bufs=1, space="DRAM") as dram,
        ):
            # --- persistent SBUF tensors ---
            wqk_sb = persist.tile([128, 8, 128], bf16, tag="wqk")
            wv_sb = persist.tile([128, 8, H], bf16, tag="wv")
            mask_sb = persist.tile([128, 2, QB], bf16, tag="mask")
            qT_own = persist.tile([64, RPC], bf16, tag="qT_own")
            qT_full = persist.tile([64, S], bf16, tag="qT_full")
            kT = persist.tile([64, RPC], bf16, tag="kT")
            vT = persist.tile([64, RPC], bf16, tag="vT")
            v_all = persist.tile([128, NKC, H + 1], bf16, tag="v_all")
            id_bf = persist.tile([128, 128], bf16, tag="id_bf")
            id_f32 = persist.tile([128, 128], f32, tag="id_f32")

            make_identity(nc, id_bf.opt())
            make_identity(nc, id_f32.opt())
            nc.vector.memset(v_all[:, :, H], 1.0)

            for dc in range(8):
                nc.sync.dma_start(out=wqk_sb[:, dc, :], in_=wqk_ext[ts(dc, 128), :])
                nc.sync.dma_start(out=wv_sb[:, dc, :], in_=wv_ext[ts(dc, 128), :])
            nc.sync.dma_start(out=mask_sb[:, 0, :], in_=mask_ext[0])
            nc.sync.dma_start(out=mask_sb[:, 1, :], in_=mask_ext[1])

            # --- phase 1: projections over own rows ---
            with (
                tc.tile_pool(name="xt", bufs=2) as xt_pool,
                tc.tile_pool(name="pj", bufs=2, space="PSUM") as pj_pool,
                tc.tile_pool(name="pv", bufs=2, space="PSUM") as pv_pool,
            ):
                for sg in range(NSG):
                    xT = xt_pool.tile([128, 8, QB], bf16, tag="xT")
                    for dc in range(8):
                        nc.sync.dma_start(
                            out=xT[:, dc, :],
                            in_=x_ext[ts(sg, QB), ts(dc, 128)],
                            transpose=True,
                        )
                    qk_ps = pj_pool.tile([128, QB], f32, tag="qk")
                    for dc in range(8):
                        nc.tensor.matmul(
                            qk_ps[:],
                            lhsT=wqk_sb[:, dc, :],
                            rhs=xT[:, dc, :],
                            start=(dc == 0),
                            stop=(dc == 7),
                        )
                    nc.any.tensor_copy(qT_own[:, ts(sg, QB)], qk_ps[0:64, :])
                    nc.any.tensor_copy(kT[:, ts(sg, QB)], qk_ps[64:128, :])
                    v_ps = pv_pool.tile([64, QB], f32, tag="v")
                    for dc in range(8):
                        nc.tensor.matmul(
                            v_ps[:],
                            lhsT=wv_sb[:, dc, :],
                            rhs=xT[:, dc, :],
                            start=(dc == 0),
                            stop=(dc == 7),
                        )
                    nc.any.tensor_copy(vT[:, ts(sg, QB)], v_ps[:])

            # --- phase 2: Q^T pair AllGather; V^T -> V natural transposes ---
            q_bounce = dram.tile([64, RPC], bf16, tag="q_bounce")
            q_gath = dram.tile([2, 64, RPC], bf16, tag="q_gath")
            nc.sync.dma_start(out=q_bounce[:], in_=qT_own[:])
            nc.gpsimd.collective_compute(
                "AllGather",
                Alu.bypass,
                replica_groups=PAIRS,
                ins=[q_bounce.opt()],
                outs=[q_gath.opt()],
            )

            with tc.tile_pool(name="vt_ps", bufs=2, space="PSUM") as vt_ps_pool:
                for i in range(NKC):
                    tp = vt_ps_pool.tile([128, H], bf16, tag="tp")
                    nc.tensor.transpose(tp[:], vT[:, ts(i, 128)], id_bf[0:64, 0:64])
                    nc.any.tensor_copy(v_all[:, i, 0:H], tp[:])

            # gathered Q^T -> SBUF in global column order:
            # global chunk c lives at gathered slot c%2, local offset (c//2)*128
            for c in range(S // 128):
                nc.sync.dma_start(
                    out=qT_full[:, ts(c, 128)],
                    in_=q_gath[c % 2, :, ts(c // 2, 128)],
                )

            # --- phase 3: attention ---
            merge = dram.tile([S, H + 1], f32, tag="merge")
            with (
                tc.tile_pool(name="st", bufs=2, space="PSUM") as st_pool,
                tc.tile_pool(name="av", bufs=2, space="PSUM") as av_pool,
                tc.tile_pool(name="tr", bufs=2, space="PSUM") as tr_pool,
                tc.tile_pool(name="p", bufs=3) as p_pool,
                tc.tile_pool(name="o", bufs=2) as o_pool,
                tc.tile_pool(name="m", bufs=3) as m_pool,
            ):
                for t in range(NQB):
                    E = 2 * (t + 1)  # local causal chunk count
                    av_ps = av_pool.tile([H + 1, QB], f32, tag="av")
                    for i in range(E):
                        st_ps = st_pool.tile([128, QB], f32, tag="st")
                        nc.tensor.matmul(
                            st_ps[:],
                            lhsT=kT[:, ts(i, 128)],
                            rhs=qT_full[:, ts(t, QB)],
                            start=True,
                            stop=True,
                        )
                        p_sb = p_pool.tile([128, QB], bf16, tag="p")
                        nc.scalar.activation(p_sb[:], st_ps[:], Act.Exp, scale=0.125)
                        if i >= E - 2:
                            j = i - (E - 2)
                            nc.vector.tensor_tensor(
                                p_sb[:], p_sb[:], mask_sb[:, j, :], Alu.mult
                            )
                        nc.tensor.matmul(
                            av_ps[:],
                            lhsT=v_all[:, i, :],
                            rhs=p_sb[:],
                            start=(i == 0),
                            stop=(i == E - 1),
                        )
                    o_sb = o_pool.tile([H + 1, QB], f32, tag="o")
                    nc.any.tensor_copy(o_sb[:], av_ps[:])
                    for a in range(4):
                        tr_ps = tr_pool.tile([128, H + 1], f32, tag="tr")
                        nc.tensor.transpose(
                            tr_ps[:], o_sb[:, ts(a, 128)], id_f32[0 : H + 1, 0 : H + 1]
                        )
                        m_sb = m_pool.tile([128, H + 1], f32, tag="m")
                        nc.vector.tensor_copy(m_sb[:], tr_ps[:])
                        nc.sync.dma_start(
                            out=merge[t * QB + a * 128 : t * QB + (a + 1) * 128, :],
                            in_=m_sb[:],
                        )

            # --- phase 4: pair ReduceScatter + output ---
            rs_out = dram.tile([RPC, H + 1], f32, tag="rs_out")
            nc.gpsimd.collective_compute(
                "ReduceScatter",
                Alu.add,
                replica_groups=PAIRS,
                ins=[merge.opt()],
                outs=[rs_out.opt()],
            )
            nc.sync.dma_start(out=out_ext[:], in_=rs_out[:])

    nc.finalize()
    return nc


def _make_masks(g: int) -> np.ndarray:
    # mask[j][kk, qq] = 1 if query (512t + qq) >= key 128*(4t + 2j + g) + kk
    m = np.zeros((2, 128, QB), dtype=np.float32)
    for j in range(2):
        dk = 128 * (2 * j + g) + np.arange(128)[:, None]
        dq = np.arange(QB)[None, :]
        m[j] = (dq >= dk).astype(np.float32)
    return m.astype(BF16)


def _shard_inputs(input, Wq, Wk, Wv):
    x = np.ascontiguousarray(input)
    wqk = np.concatenate([Wq, Wk], axis=1).astype(BF16)
    wv = np.ascontiguousarray(Wv).astype(BF16)
    masks = [_make_masks(0), _make_masks(1)]
    in_maps = []
    for c in range(8):
        b, g = c // 2, c % 2
        xs = x[b].reshape(S // 128, 128, D)[g::2].reshape(RPC, D).astype(BF16)
        in_maps.append(
            {"x": np.ascontiguousarray(xs), "wqk": wqk, "wv": wv, "mask": masks[g]}
        )
    return in_maps


def _unshard(results):
    out = np.empty((B, S, H), dtype=np.float32)
    for b in range(B):
        merged = np.concatenate(
            [results[2 * b]["out"], results[2 * b + 1]["out"]], axis=0
        )
        out[b] = merged[:, :H] / merged[:, H : H + 1]
    return out


def _run(inputs, trace=False):
    from concourse.bass_utils import run_bass_kernel_spmd

    if "nc" not in _CACHE:
        _CACHE["nc"] = _build()
    nc = _CACHE["nc"]
    in_maps = _shard_inputs(**inputs)
    res = run_bass_kernel_spmd(nc, in_maps, core_ids=list(range(8)), trace=trace)
    out = _unshard(res.results)
    return out, res


def kernel(**inputs) -> np.ndarray:
    out, _ = _run(inputs, trace=False)
    return out



# revision 4
# speedup vs baseline: 1.8672x; 1.8672x over previous
"""Distributed causal attention head for TRN2 (8 NeuronCores), v2.

Problem: B=4, S=4096, D=1024, H=64 fp32.
  q,k,v = x @ W{q,k,v}; scores = q k^T / sqrt(H); causal softmax; out = P v.

Design (collective-free, one SPMD-uniform NEFF, no barrier/AllGather/RS):
  - 4 batches x 2 cores per batch. Each core receives the FULL batch x^T
    (bf16, host-pretransposed) and projects Q^T for ALL 4096 queries
    itself; K/V only for the 2048 interleaved key rows it owns
    (128-chunk interleave keeps the causal work perfectly balanced).
  - SPMD uniformity: the host permutes x^T columns per core so the
    core's OWN key chunks sit at even 128-chunk positions. All extraction
    addresses are then identical across cores; causality differences are
    absorbed into per-core 0/1 mask DATA (queries are consistently in the
    permuted order; the host un-permutes the output).
  - Projections run dc-outer (stationary [Wq|Wk] loaded once per dc
    chunk, 4 moving blocks of 512) to amortize LDWEIGHTS; V is computed
    NATURALLY ([keys,64], x^T chunk stationary, Wv moving) so no PE
    transposes are needed.
  - Attention is emitted as two lagged streams so the Tensor engine never
    waits on the Scalar engine (PE DVFS: full 2.4 GHz only after ~3us of
    continuous execution): score matmuls fill [128,1024] PSUM tiles
    (chunk-PAIRS), the Scalar engine exps them into a big persistent
    SBUF P buffer (72KB/partition), and AV matmuls consume P two slots
    behind. V is augmented with a ones column so AV also emits the
    softmax denominator.
  - Per-core partial (num^T | den) = [65, 4096] f32 goes straight to
    DRAM; the HOST adds the two partials of each pair, divides, and
    transposes. No on-device collective at all.
"""

import sys

sys.path.insert(0, "/opt/trn_rl_repo")

import numpy as np
import ml_dtypes

B, S, D, H = 4, 4096, 1024, 64
RPC = S // 2            # key rows owned per core
QB = 512                # query block width
NQB = S // QB           # 8 query blocks
NKC = RPC // 128        # 16 local key chunks
BF16 = ml_dtypes.bfloat16

_CACHE = {}


def _build():
    import concourse.bass as bass
    import concourse.mybir as mybir
    from concourse import bacc, tile
    from concourse.bass import ts

    f32 = mybir.dt.float32
    bf16 = mybir.dt.bfloat16
    Alu = mybir.AluOpType
    Act = mybir.ActivationFunctionType

    nc = bacc.Bacc(None, target_bir_lowering=False)

    xt_ext = nc.declare_dram_parameter("xt", [D, S], bf16, isOutput=False)
    wqk_ext = nc.declare_dram_parameter("wqk", [D, 128], bf16, isOutput=False)
    wv_ext = nc.declare_dram_parameter("wv", [D, H], bf16, isOutput=False)
    mask_ext = nc.declare_dram_parameter("mask", [128, 1024], bf16, isOutput=False)
    out_ext = nc.declare_dram_parameter("out", [H + 1, S], f32, isOutput=True)

    with tile.TileContext(nc) as tc:
        with tc.tile_pool(name="persist", bufs=1) as persist:
            # --- persistent SBUF tensors ---
            wqk_sb = persist.tile([128, 8, 128], bf16, tag="wqk")
            wv_sb = persist.tile([128, 8, H], bf16, tag="wv")
            mask_sb = persist.tile([128, 1024], bf16, tag="mask")
            qT = persist.tile([64, S], bf16, tag="qT")
            kT = persist.tile([64, RPC], bf16, tag="kT")
            v_all = persist.tile([128, NKC, H + 1], bf16, tag="v_all")
            p_sb = persist.tile([128, 36, 1024], bf16, tag="p")

            nc.vector.memset(v_all[:, :, H], 1.0)

            for dc in range(8):
                nc.sync.dma_start(out=wqk_sb[:, dc, :], in_=wqk_ext[ts(dc, 128), :])
                nc.sync.dma_start(out=wv_sb[:, dc, :], in_=wv_ext[ts(dc, 128), :])
            nc.sync.dma_start(out=mask_sb[:], in_=mask_ext[:])

            # --- phase 1: projections (dc-outer, half-S at a time) ---
            with (
                tc.tile_pool(name="xt", bufs=2) as xt_pool,
                tc.tile_pool(name="pj", bufs=4, space="PSUM") as pj_pool,
                tc.tile_pool(name="pv", bufs=2, space="PSUM") as pv_pool,
            ):
                xts = []
                for hf in range(2):
                    xth = xt_pool.tile([128, 8, S // 2], bf16, tag="xt")
                    xts.append(xth)
                    for dc in range(8):
                        eng = nc.sync if dc % 2 == 0 else nc.scalar
                        eng.dma_start(
                            out=xth[:, dc, :],
                            in_=xt_ext[ts(dc, 128), ts(hf, S // 2)],
                        )

                for hf in range(2):
                    # Q|K projection for this half: stationary [Wq|Wk] per
                    # dc chunk, 4 moving blocks of 512 queries.
                    qkps = [
                        pj_pool.tile([128, QB], f32, tag="qk", name=f"qk{s}")
                        for s in range(4)
                    ]
                    for dc in range(8):
                        for s in range(4):
                            nc.tensor.matmul(
                                qkps[s][:],
                                lhsT=wqk_sb[:, dc, :],
                                rhs=xts[hf][:, dc, ts(s, QB)],
                                start=(dc == 0),
                                stop=(dc == 7),
                                skip_group_check=True,
                            )
                    # extraction (DVE): Q^T all columns; K^T from the two
                    # OWN chunks (even 128-chunk positions) of each block.
                    for s in range(4):
                        sg = 4 * hf + s
                        nc.vector.tensor_copy(qT[:, ts(sg, QB)], qkps[s][0:64, :])
                        nc.vector.tensor_copy(
                            kT[:, 256 * sg : 256 * sg + 128],
                            qkps[s][64:128, 0:128],
                        )
                        nc.vector.tensor_copy(
                            kT[:, 256 * sg + 128 : 256 * sg + 256],
                            qkps[s][64:128, 256:384],
                        )

                # V natural: per own key chunk (128 rows), x^T chunk is the
                # stationary, Wv moves (N=64); accumulate over dc.
                for i in range(NKC):
                    hf, ii = i // 8, i % 8
                    vps = pv_pool.tile([128, H], f32, tag="v")
                    for dc in range(8):
                        nc.tensor.matmul(
                            vps[:],
                            lhsT=xts[hf][:, dc, 256 * ii : 256 * ii + 128],
                            rhs=wv_sb[:, dc, :],
                            start=(dc == 0),
                            stop=(dc == 7),
                        )
                    nc.vector.tensor_copy(v_all[:, i, 0:H], vps[:])

            # --- phase 2: attention, two lagged streams ---
            pairs = [(t, ip) for t in range(NQB) for ip in range(t + 1)]
            LAG = 2
            av_tiles = {}
            with (
                tc.tile_pool(name="st", bufs=3, space="PSUM") as st_pool,
                tc.tile_pool(name="av", bufs=2, space="PSUM") as av_pool,
                tc.tile_pool(name="o", bufs=2) as o_pool,
            ):
                for slot in range(len(pairs) + LAG):
                    if slot < len(pairs):
                        t, ip = pairs[slot]
                        st2 = st_pool.tile([128, 1024], f32, tag="st")
                        nc.tensor.matmul(
                            st2[:, 0:512],
                            lhsT=kT[:, 256 * ip : 256 * ip + 128],
                            rhs=qT[:, ts(t, QB)],
                            start=True,
                            stop=True,
                            skip_group_check=True,
                        )
                        nc.tensor.matmul(
                            st2[:, 512:1024],
                            lhsT=kT[:, 256 * ip + 128 : 256 * ip + 256],
                            rhs=qT[:, ts(t, QB)],
                            start=True,
                            stop=True,
                            skip_group_check=True,
                        )
                        nc.scalar.activation(
                            p_sb[:, slot, :], st2[:], Act.Exp, scale=0.125
                        )
                        if ip == t:  # diagonal chunk pair: multiplicative mask
                            nc.vector.tensor_tensor(
                                p_sb[:, slot, :], p_sb[:, slot, :], mask_sb[:], Alu.mult
                            )
                    k = slot - LAG
                    if k >= 0:
                        t, ip = pairs[k]
                        if ip == 0:
                            av_tiles[t] = av_pool.tile(
                                [H + 1, QB], f32, tag="av", name=f"av{t}"
                            )
                        av = av_tiles[t]
                        nc.tensor.matmul(
                            av[:],
                            lhsT=v_all[:, 2 * ip, :],
                            rhs=p_sb[:, k, 0:512],
                            start=(ip == 0),
                            stop=False,
                            skip_group_check=True,
                        )
                        nc.tensor.matmul(
                            av[:],
                            lhsT=v_all[:, 2 * ip + 1, :],
                            rhs=p_sb[:, k, 512:1024],
                            start=False,
                            stop=(ip == t),
                            skip_group_check=True,
                        )
                        if ip == t:
                            o_sb = o_pool.tile([H + 1, QB], f32, tag="o")
                            nc.vector.tensor_copy(o_sb[:], av[:])
                            nc.sync.dma_start(
                                out=out_ext[:, ts(t, QB)], in_=o_sb[:]
                            )

    nc.finalize()
    return nc


def _make_mask2(g: int) -> np.ndarray:
    """[128, 1024] multiplicative mask for the diagonal chunk pair of any
    query block t (t-independent thanks to the per-core permutation).

    Query columns are in permuted order: position pc in the block maps to
    global query chunk offsets delta = [g, 1-g, 2+g, 3-g] (relative to 4t).
    Left half masks own key chunk at global offset g; right half offset 2+g.
    """
    m = np.zeros((128, 1024), dtype=np.float32)
    delta = [g, 1 - g, 2 + g, 3 - g]
    kk = np.arange(128)[:, None]
    qq = np.arange(128)[None, :]
    for half, keyoff in ((0, g), (1, 2 + g)):
        for pc in range(4):
            keep = (128 * (delta[pc] - keyoff) + qq) >= kk
            m[:, half * 512 + pc * 128 : half * 512 + (pc + 1) * 128] = keep
    return m.astype(BF16)


def _swap_pairs(a: np.ndarray) -> np.ndarray:
    """Swap adjacent 128-column chunks (self-inverse permutation)."""
    n = a.shape[-1]
    return np.ascontiguousarray(
        a.reshape(a.shape[:-1] + (n // 256, 2, 128))[..., ::-1, :].reshape(a.shape)
    )


def _shard_inputs(input, Wq, Wk, Wv):
    wqk = np.ascontiguousarray(np.concatenate([Wq, Wk], axis=1)).astype(BF16)
    wv = np.ascontiguousarray(Wv).astype(BF16)
    masks = [_make_mask2(0), _make_mask2(1)]
    in_maps = []
    for b in range(B):
        xt = np.ascontiguousarray(np.asarray(input)[b].T).astype(BF16)
        xt_sw = _swap_pairs(xt)
        for g in range(2):
            in_maps.append(
                {
                    "xt": xt if g == 0 else xt_sw,
                    "wqk": wqk,
                    "wv": wv,
                    "mask": masks[g],
                }
            )
    return in_maps


def _unshard(results):
    out = np.empty((B, S, H), dtype=np.float32)
    for b in range(B):
        r0 = results[2 * b]["out"]                      # [65, S] natural order
        r1 = _swap_pairs(results[2 * b + 1]["out"])     # un-permute g=1
        m = r0 + r1
        out[b] = (m[:H] / m[H : H + 1]).T
    return out


def _run(inputs, trace=False):
    from concourse.bass_utils import run_bass_kernel_spmd

    if "nc" not in _CACHE:
        _CACHE["nc"] = _build()
    nc = _CACHE["nc"]
    in_maps = _shard_inputs(**inputs)
    res = run_bass_kernel_spmd(nc, in_maps, core_ids=list(range(8)), trace=trace)
    out = _unshard(res.results)
    return out, res


def kernel(**inputs) -> np.ndarray:
    out, _ = _run(inputs, trace=False)
    return out
